# revision 27
# baseline (speedup 1.0000x reference)
"""Multi-head self-attention prefill (B=2, S=2048, E=2048, H=16, D=128) on 8 trn2 cores.

Sharding: core c -> batch b = c//4, head-group g = c%4 (heads 4g..4g+3).
Each core computes q/k/v projections for its 4 heads (column shard of Wq/Wk/Wv),
causal attention with RoPE, and a partial output projection (row shard of Wo).
Host sums the 4 partials per batch (all-reduce equivalent) and stacks batches.

v2: bf16 matmuls (fp32 PSUM accum), host-side x transpose + packed weight
layouts (single big DMAs, weights loaded once), 1024-wide moving operands,
causal-tight ctx accumulation, copies spread across scalar/vector engines.
"""
import sys
sys.path.insert(0, "/opt/trn_rl_repo")
import numpy as np
from ml_dtypes import bfloat16

import concourse.bass as bass
import concourse.mybir as mybir
import concourse.tile as tile
from concourse import bacc
from concourse.bass import ds, ts
from concourse.masks import make_identity, make_causal_mask
from concourse.bass_utils import run_bass_kernel_spmd

S = 2048          # sequence length (per batch)
E = 2048          # embedding dim
H = 16            # total heads
D = 128           # head dim
HG = 4            # heads per core
DG = HG * D       # 512: per-core projection width
NE = E // 128     # 16 contraction chunks
NTP = 2           # token super-blocks of 1024
TP = S // NTP     # 1024
NTT = S // 128    # 16 token tiles of 128
NQC = 4           # q-chunks of 512
ROPE_BASE = 10000.0
MASK_VAL = -1e30

f32 = mybir.dt.float32
bf16 = mybir.dt.bfloat16

_CACHE = {}
DEBUG = False


def build():
    nc = bacc.Bacc(None)
    # host-packed layouts (see kernel() for packing):
    #   xt:  [256, 16*1024]  xt[tp*128+p, e*1024+t] = x[tp*1024+t, e*128+p]
    #   wq/wk/wv: [128, 16*512]  w[p, e*512+d] = W[e*128+p, d]
    #   wo:  [128, 4*2048]   wo[p, h*2048+eo] = Wo[h*128+p, eo]
    xt_in = nc.dram_tensor("xt", [NTP * 128, NE * TP], bf16, kind="ExternalInput")
    wq_in = nc.dram_tensor("wq", [128, NE * DG], bf16, kind="ExternalInput")
    wk_in = nc.dram_tensor("wk", [128, NE * DG], bf16, kind="ExternalInput")
    wv_in = nc.dram_tensor("wv", [128, NE * DG], bf16, kind="ExternalInput")
    wo_in = nc.dram_tensor("wo", [128, HG * E], bf16, kind="ExternalInput")
    cos_in = nc.dram_tensor("cosT", [128, S], bf16, kind="ExternalInput")
    sin_in = nc.dram_tensor("sinT", [128, S], bf16, kind="ExternalInput")
    out_d = nc.dram_tensor("out", [S, E], bf16, kind="ExternalOutput")
    if DEBUG:
        dbg_q = nc.dram_tensor("dbg_q", [128, S], bf16, kind="ExternalOutput")
        dbg_k = nc.dram_tensor("dbg_k", [128, S], bf16, kind="ExternalOutput")
        dbg_v = nc.dram_tensor("dbg_v", [128, DG], bf16, kind="ExternalOutput")
        dbg_ct = nc.dram_tensor("dbg_ct", [128, DG], bf16, kind="ExternalOutput")
        dbg_et = nc.dram_tensor("dbg_et", [128, 1024], bf16, kind="ExternalOutput")

    with tile.TileContext(nc) as tc:
        with tc.tile_pool(name="persist", bufs=1) as pp:
            # persistent across phases
            qT = [pp.tile([128, S], bf16, tag=f"qT{h}", name=f"qT{h}") for h in range(HG)]
            kT = [pp.tile([128, S], bf16, tag=f"kT{h}", name=f"kT{h}") for h in range(HG)]
            v_sb = [pp.tile([128, DG], bf16, tag=f"v{tt}", name=f"v{tt}") for tt in range(NTT)]
            # transposed causal mask: maskTT[k, q] = 0 if q >= k else MASK_VAL
            maskTT = pp.tile([128, 128], f32, tag="maskTT")
            nc.gpsimd.memset(maskTT[:], 0.0)
            nc.gpsimd.affine_select(
                out=maskTT[:], in_=maskTT[:],
                compare_op=mybir.AluOpType.is_ge, fill=MASK_VAL,
                base=0, pattern=[[1, 128]], channel_multiplier=-1)
            zero_bf = pp.tile([128, 512], bf16, tag="zero_bf")
            nc.gpsimd.memset(zero_bf[:], 0.0)
            ones_sb = pp.tile([128, 128], bf16, tag="ones_sb")
            nc.gpsimd.memset(ones_sb[:], 1.0)

            # ---------------- Phase A: projections + RoPE ----------------
            with tc.tile_pool(name="phA", bufs=1) as pa, \
                 tc.tile_pool(name="phA2", bufs=2) as pa2, \
                 tc.tile_pool(name="psQK", bufs=2, space="PSUM") as psQK, \
                 tc.tile_pool(name="psV", bufs=2, space="PSUM") as psV:
                # weights + first x block, interleaved so the first q-proj
                # accumulation chain can start as soon as slices land
                wq_sb = pa.tile([128, NE * DG], bf16, tag="wq")
                wk_sb = pa.tile([128, NE * DG], bf16, tag="wk")
                wv_sb = pa.tile([128, NE * DG], bf16, tag="wv")
                xTs0 = pa2.tile([128, NE * TP], bf16, tag="xT", name="xTs0")
                for j in range(8):
                    nc.sync.dma_start(out=wq_sb[:, ts(j, 1024)], in_=wq_in[:, ts(j, 1024)])
                    nc.sync.dma_start(out=xTs0[:, ts(j, 2048)],
                                      in_=xt_in[0:128, ts(j, 2048)])
                for j in range(4):
                    nc.sync.dma_start(out=wv_sb[:, ts(j, 2048)], in_=wv_in[:, ts(j, 2048)])
                cosT = pa.tile([128, S], bf16, tag="cos")
                nc.sync.dma_start(out=cosT[:], in_=cos_in[:])
                sinT = pa.tile([128, S], bf16, tag="sin")
                nc.sync.dma_start(out=sinT[:], in_=sin_in[:])
                for j in range(4):
                    nc.sync.dma_start(out=wk_sb[:, ts(j, 2048)], in_=wk_in[:, ts(j, 2048)])

                for tp in range(NTP):
                    if tp == 0:
                        xTs = xTs0
                    else:
                        xTs = pa2.tile([128, NE * TP], bf16, tag="xT", name="xT")
                        for j in range(4):
                            nc.sync.dma_start(
                                out=xTs[:, ts(j, NE * TP // 4)],
                                in_=xt_in[ds(tp * 128, 128), ts(j, NE * TP // 4)])

                    # projections ordered q, v, k so the wv/wk DMAs have time to
                    # land while earlier GEMMs run (moving dim capped at 512 by
                    # the ISA -> two 512 chains per 1024-token block)
                    def qk_proj(w_sb, dstT):
                        for h in range(HG):
                            ps = psQK.tile([128, TP], f32, tag="pqk", name="pqk")
                            for e in range(NE):
                                for hf in range(2):
                                    nc.tensor.matmul(
                                        ps[:, ts(hf, 512)],
                                        w_sb[:, ds(e * DG + h * 128, 128)],
                                        xTs[:, ds(e * TP + hf * 512, 512)],
                                        start=(e == 0), stop=(e == NE - 1))
                            sl = dstT[h][:, ts(tp, TP)]
                            cs = cosT[:, ts(tp, TP)]
                            sn = sinT[:, ts(tp, TP)]
                            # RoPE: sl = raw*cos + swap(raw)*sin  (sin signed +-)
                            nc.scalar.copy(sl, ps[:])
                            swp = pa2.tile([128, TP], bf16, tag="swp", name="swp")
                            nc.sync.dma_start(out=swp[0:64, :],
                                              in_=dstT[h][64:128, ts(tp, TP)])
                            nc.sync.dma_start(out=swp[64:128, :],
                                              in_=dstT[h][0:64, ts(tp, TP)])
                            nc.vector.tensor_mul(swp[:], swp[:], sn)
                            nc.vector.tensor_mul(sl, sl, cs)
                            nc.vector.tensor_add(sl, sl, swp[:])

                    qk_proj(wq_sb, qT)
                    # v projection: stationary = xT chunk, moving = Wv chunk
                    for t8 in range(8):
                        tt = tp * 8 + t8
                        ps = psV.tile([128, DG], f32, tag="pv", name="pv")
                        for e in range(NE):
                            nc.tensor.matmul(ps[:], xTs[:, ds(e * TP + t8 * 128, 128)],
                                             wv_sb[:, ts(e, DG)],
                                             start=(e == 0), stop=(e == NE - 1))
                        (nc.vector.tensor_copy if t8 % 2 else nc.scalar.copy)(
                            v_sb[tt][:], ps[:])
                    qk_proj(wk_sb, kT)

            if DEBUG:
                nc.sync.dma_start(out=dbg_q[:], in_=qT[0][:])
                nc.sync.dma_start(out=dbg_k[:], in_=kT[0][:])
                nc.sync.dma_start(out=dbg_v[:], in_=v_sb[4][:])

            # ---------------- Phase B: attention + output projection ----------------
            # scores computed TRANSPOSED (S^T[k, q] via stationary=kT chunk), so no
            # PE transposes / PSUM->SBUF attn copies. z comes from an all-ones
            # stationary matmul (z replicated across partitions); normalization is
            # fused into the ctx PSUM->SBUF copy.
            with tc.tile_pool(name="phB", bufs=1) as pb, \
                 tc.tile_pool(name="phB3", bufs=3) as pb3, \
                 tc.tile_pool(name="psS", bufs=2, space="PSUM") as psS, \
                 tc.tile_pool(name="psZ", bufs=2, space="PSUM") as psZ, \
                 tc.tile_pool(name="psCO", bufs=2, space="PSUM") as psCO:
                wo_sb = pb.tile([128, HG * E], bf16, tag="wo")
                for j in range(4):
                    nc.sync.dma_start(out=wo_sb[:, ts(j, 2048)], in_=wo_in[:, ts(j, 2048)])

                ctxT = {}

                def st_one(qc, h, kt):
                    """Emit S^T + mask + exp + zero-fill for one kt; return et."""
                    zc = max(0, kt - 4 * qc)   # first valid q4
                    pst = psS.tile([128, 512], f32, tag="pst", name="pst", bufs=4)
                    et = pb3.tile([128, 512], bf16, tag="et", name="et", bufs=6)
                    w = 512 - zc * 128
                    nc.tensor.matmul(pst[:, ds(zc * 128, w)],
                                     kT[h][:, ts(kt, 128)],
                                     qT[h][:, ds(qc * 512 + zc * 128, w)],
                                     start=True, stop=True)
                    if kt >= 4 * qc:  # diagonal tile: q4 == zc
                        sl = pst[:, ds(zc * 128, 128)]
                        nc.vector.tensor_add(sl, sl, maskTT[:])
                    nc.scalar.activation(et[:, ds(zc * 128, w)],
                                         pst[:, ds(zc * 128, w)],
                                         mybir.ActivationFunctionType.Exp)
                    if zc > 0:
                        nc.vector.tensor_copy(et[:, 0:zc * 128],
                                              zero_bf[:, 0:zc * 128])
                    if DEBUG and qc == 1 and h == 0 and kt in (2, 3):
                        nc.sync.dma_start(out=dbg_et[:, ts(kt - 2, 512)], in_=et[:])
                    return et

                last_et = {}

                def consume(p):
                    """Emit z + ctx matmuls for a pending kt; epilogue on last.

                    z is accumulated per kt-PAIR: the two et tiles are summed on
                    DVE first, halving the z matmul count. ctx matmuls read only
                    the causally-valid span (zero-filled spans feed only z)."""
                    qc, h, kt, et, pc, zp, nkt = p
                    if kt % 2 == 0:
                        last_et[(qc, h)] = et
                    else:
                        es = pb3.tile([128, 512], bf16, tag="es", name="es", bufs=3)
                        nc.vector.tensor_add(es[:], last_et[(qc, h)][:], et[:])
                        nc.tensor.matmul(zp[:], ones_sb[:], es[:],
                                         start=(kt == 1), stop=(kt == nkt - 1))
                    zc = max(0, kt - 4 * qc)
                    nc.tensor.matmul(pc[:, ds(zc * 128, 512 - zc * 128)],
                                     v_sb[kt][:, ts(h, 128)],
                                     et[:, ds(zc * 128, 512 - zc * 128)],
                                     start=(kt == 0), stop=(kt == nkt - 1),
                                     skip_group_check=(zc > 0))
                    if kt == nkt - 1:
                        rzb = pb3.tile([128, 512], f32, tag="rzb", name="rzb", bufs=2)
                        nc.vector.reciprocal_approx_fast(rzb[:], zp[:])
                        ct = pb3.tile([128, 512], bf16, tag=f"ctxT{h}",
                                      name=f"ctxT{h}", bufs=2)
                        nc.vector.tensor_mul(ct[:], pc[:], rzb[:])
                        ctxT[(qc, h)] = ct
                        if DEBUG and qc == 1 and h == 0:
                            nc.sync.dma_start(out=dbg_ct[:], in_=ct[:])

                def emit_op(qc):
                    """Output projection for q-chunk qc (needs ctxT[(qc, 0..3)])."""
                    for t4 in range(4):
                        row0 = qc * 512 + t4 * 128
                        ob = pb3.tile([128, E], bf16, tag="ob", name="ob", bufs=2)
                        for e4 in range(4):
                            po = psCO.tile([128, 512], f32, tag="pco", name="po")
                            for h in range(HG):
                                nc.tensor.matmul(po[:],
                                                 ctxT[(qc, h)][:, ts(t4, 128)],
                                                 wo_sb[:, ds(h * E + e4 * 512, 512)],
                                                 start=(h == 0), stop=(h == HG - 1))
                            (nc.vector.tensor_copy if e4 % 2 else nc.scalar.copy)(
                                ob[:, ts(e4, 512)], po[:])
                        nc.sync.dma_start(out=out_d[ds(row0, 128), :], in_=ob[:])

                # pipeline: z/ctx consumption lags S^T/exp by LAG kts, carrying
                # across (qc, h) blocks; OP(qc) is emitted a few kts into
                # (qc+1, h0) so the last epilogue's DVE latency is hidden.
                LAG = 3
                from collections import deque
                pending = deque()
                op_queue = None
                for qc in range(NQC):
                    nkt = 4 * qc + 4        # k tiles needed for this q-chunk
                    for h in range(HG):
                        pc = psCO.tile([128, 512], f32, tag="pco", name="pc")
                        zp = psZ.tile([128, 512], f32, tag="zp", name="zp")
                        for kt in range(nkt):
                            et = st_one(qc, h, kt)
                            pending.append((qc, h, kt, et, pc, zp, nkt))
                            if len(pending) > LAG:
                                consume(pending.popleft())
                            if op_queue is not None and h == 0 and kt == 4:
                                emit_op(op_queue)
                                op_queue = None
                    op_queue = qc
                while pending:
                    consume(pending.popleft())
                emit_op(NQC - 1)
    nc.finalize()
    return nc


def _host_tables():
    half = D // 2
    inv = 1.0 / (ROPE_BASE ** (np.arange(half, dtype=np.float64) * 2.0 / D))
    ang = np.arange(S, dtype=np.float64)[None, :] * inv[:, None]   # [64, S]
    cos = np.cos(ang)
    sin = np.sin(ang)
    cosT = np.concatenate([cos, cos], axis=0)                      # [128, S]
    sinT = np.concatenate([-sin, sin], axis=0)                     # [128, S]
    return cosT.astype(bfloat16), sinT.astype(bfloat16)


def kernel(x, start_pos, Wq, Wk, Wv, Wo):
    x = np.asarray(x, dtype=np.float32)
    Wq = np.asarray(Wq, dtype=np.float32)
    Wk = np.asarray(Wk, dtype=np.float32)
    Wv = np.asarray(Wv, dtype=np.float32)
    Wo = np.asarray(Wo, dtype=np.float32)
    B = x.shape[0]
    assert x.shape == (B, S, E) and B == 2

    cosT, sinT = _host_tables()
    perm = np.concatenate([np.arange(0, D, 2), np.arange(1, D, 2)])
    scale = 1.0 / np.sqrt(D)

    def pack_w(w):  # [E, DG] -> [128, NE*DG]
        return np.ascontiguousarray(
            w.reshape(NE, 128, DG).transpose(1, 0, 2).reshape(128, NE * DG))

    in_maps = []
    for c in range(8):
        b, g = c // 4, c % 4
        cols = slice(DG * g, DG * g + DG)
        wq = (Wq[:, cols] * scale).reshape(E, HG, D)[:, :, perm].reshape(E, DG)
        wk = Wk[:, cols].reshape(E, HG, D)[:, :, perm].reshape(E, DG)
        wv = Wv[:, cols]
        # xt[tp*128+p, e*1024+t] = x[b, tp*1024+t, e*128+p]
        xt = (x[b].reshape(NTP, TP, NE, 128)      # [tp, t, e, p]
              .transpose(0, 3, 2, 1)              # [tp, p, e, t]
              .reshape(NTP * 128, NE * TP))
        # wo[p, h*2048+eo] = Wo[g*DG + h*128 + p, eo]
        wo = (Wo[cols, :].reshape(HG, 128, E)
              .transpose(1, 0, 2).reshape(128, HG * E))
        in_maps.append({
            "xt": np.ascontiguousarray(xt).astype(bfloat16),
            "wq": pack_w(wq).astype(bfloat16),
            "wk": pack_w(wk).astype(bfloat16),
            "wv": pack_w(wv).astype(bfloat16),
            "wo": np.ascontiguousarray(wo).astype(bfloat16),
            "cosT": cosT,
            "sinT": sinT,
        })

    if "nc" not in _CACHE:
        _CACHE["nc"] = build()
    nc = _CACHE["nc"]
    _CACHE["in_maps"] = in_maps
    res = run_bass_kernel_spmd(nc, in_maps, list(range(8)))
    parts = [res.results[c]["out"].astype(np.float32) for c in range(8)]
    out = np.stack([
        parts[0] + parts[1] + parts[2] + parts[3],
        parts[4] + parts[5] + parts[6] + parts[7],
    ]).astype(np.float32)
    return out
